# revision 3
# baseline (speedup 1.0000x reference)
"""Trainium2 Bass kernel for nn_ChannelModel (cross-attention + bilinear + logsigmoid sum).

Reference computation (per full problem, N=16384, M=1024, Ds=2048):
    scores = (D @ S.T) / sqrt(Ds)            # [N, M]
    w      = softmax(scores, axis=1)         # [N, M]
    att_S  = w @ S                           # [N, Ds]
    logits[i] = D[i] . (W @ att_S[i])        # [N]
    out    = sum(log_sigmoid(logits))        # scalar

Algebraic restructuring used here:
    logits[i] = sum_j w[i,j] * B[i,j]   with   B = D @ G.T,  G.T = W @ S.T
so per core (N sharded 8 ways, 2048 rows each):
    GT   = W @ S.T                  (replicated compute, [Ds, M])
    scoresT-free pipeline: per 128-row tile of D:
        scores = D_t @ S.T          (PSUM, via lhsT = D_t.T)
        B      = D_t @ G.T          (PSUM, same lhsT)
        e = exp(scores/sqrt(Ds)), sumexp = rowsum(e)     (ACT, fused accum)
        lu = rowsum(e * B)                               (DVE, fused reduce)
    logits = lu / sumexp; partial = sum(softplus(-logits)); out = -sum(partial)
All matmul inputs are bf16 (fp32 PSUM accumulation); validated end-to-end
error of the final scalar vs fp32 reference is ~1e-5 relative.
"""

import math
import os
import sys

for _p in ("/opt/trn_rl_repo", "/root/.axon_site/_ro/trn_rl_repo"):
    if os.path.isdir(_p) and _p not in sys.path:
        sys.path.insert(0, _p)

import ml_dtypes
import numpy as np

import concourse.bass as bass
import concourse.tile as tile
from concourse import bacc, mybir
from concourse.bass_utils import run_bass_kernel_spmd

N_CORES = 8
N_FULL = 16384
M = 1024
DS = 2048
N_LOC = N_FULL // N_CORES   # 2048 rows per core
NT = N_LOC // 128           # 16 row-tiles per core
KT = DS // 128              # 16 contraction slices
JT = M // 512               # 2 free-dim halves of the M axis

BF16 = mybir.dt.bfloat16
F32 = mybir.dt.float32


def _build_program():
    nc = bacc.Bacc("TRN2", target_bir_lowering=False, debug=False,
                   num_devices=N_CORES)

    # DRAM parameters (per-core shapes; packed on host, see kernel()).
    # dtp[it, p, ds, ii] = D_shard[it*128+ii, ds*128+p]
    dt_ap = nc.dram_tensor("dtp", [NT, 128, KT, 128], BF16,
                           kind="ExternalInput").ap()
    # stp[p, ds, j] = S[j, ds*128+p]   (= S.T with the Ds axis on partitions)
    st_ap = nc.dram_tensor("stp", [128, KT, M], BF16,
                           kind="ExternalInput").ap()
    # wtp[dt, p, es, ii] = W[dt*128+ii, es*128+p]
    wt_ap = nc.dram_tensor("wtp", [KT, 128, KT, 128], BF16,
                           kind="ExternalInput").ap()
    out_ap = nc.dram_tensor("out", [1, 1], F32, kind="ExternalOutput").ap()

    scale = 1.0 / math.sqrt(DS)

    with tile.TileContext(nc) as tc:
        with (
            tc.tile_pool(name="singles", bufs=1) as singles,
            tc.tile_pool(name="wt_pool", bufs=3) as wt_pool,
            tc.tile_pool(name="dt_pool", bufs=3) as dt_pool,
            tc.tile_pool(name="e_pool", bufs=2) as e_pool,
            tc.tile_pool(name="prod_pool", bufs=2) as prod_pool,
            tc.tile_pool(name="psum", bufs=2, space="PSUM") as psum_pool,
        ):
            # Long-lived SBUF tensors.
            st_sb = singles.tile([128, KT, M], BF16)
            nc.sync.dma_start(out=st_sb[:], in_=st_ap)
            gt_sb = singles.tile([128, KT, M], BF16)
            se_buf = singles.tile([128, NT], F32)
            lu_buf = singles.tile([128, NT], F32)

            # ---- Phase 1: GT = W @ S.T, laid out like st_sb ----
            for dt_i in range(KT):
                wt_t = wt_pool.tile([128, KT, 128], BF16, tag="wt")
                nc.sync.dma_start(out=wt_t[:], in_=wt_ap[dt_i])
                pg = psum_pool.tile([128, M], F32, tag="s")
                for jh in range(JT):
                    js = slice(jh * 512, (jh + 1) * 512)
                    for es in range(KT):
                        nc.tensor.matmul(
                            pg[:, js], wt_t[:, es, :], st_sb[:, es, js],
                            start=(es == 0), stop=(es == KT - 1),
                        )
                nc.vector.tensor_copy(gt_sb[:, dt_i, :], pg[:])

            # ---- Phase 2: per 128-row tile: scores, B, exp, reduce ----
            for it in range(NT):
                dt_t = dt_pool.tile([128, KT, 128], BF16, tag="dt")
                nc.sync.dma_start(out=dt_t[:], in_=dt_ap[it])
                ps = psum_pool.tile([128, M], F32, tag="s")
                pb = psum_pool.tile([128, M], F32, tag="b")
                for ds in range(KT):
                    lhsT = dt_t[:, ds, :]
                    for jh in range(JT):
                        js = slice(jh * 512, (jh + 1) * 512)
                        nc.tensor.matmul(ps[:, js], lhsT, st_sb[:, ds, js],
                                         start=(ds == 0), stop=(ds == KT - 1))
                        nc.tensor.matmul(pb[:, js], lhsT, gt_sb[:, ds, js],
                                         start=(ds == 0), stop=(ds == KT - 1))
                e_t = e_pool.tile([128, M], F32, tag="e")
                nc.scalar.activation(
                    out=e_t[:], in_=ps[:],
                    func=mybir.ActivationFunctionType.Exp,
                    scale=scale, accum_out=se_buf[:, it:it + 1],
                )
                prod_t = prod_pool.tile([128, M], F32, tag="p")
                nc.vector.tensor_mul(prod_t[:], pb[:], e_t[:])
                nc.vector.reduce_sum(lu_buf[:, it:it + 1], prod_t[:],
                                     mybir.AxisListType.X)

            # ---- Epilogue: logits -> sum(softplus(-logits)) -> scalar ----
            # softplus(-x) = ln(z), z = 1 + exp(-x), via initial guess
            # relu(-x) + ln2*exp(-0.7213*|x|) and 2 Newton steps
            # y <- y - 1 + z*exp(-y). Stays within the Exp/Relu/Copy ACT
            # table (no Softplus/Ln table exists on this build).
            LN2 = 0.6931471805599453
            Exp = mybir.ActivationFunctionType.Exp
            Relu = mybir.ActivationFunctionType.Relu
            rse = singles.tile([128, NT], F32)
            nc.vector.reciprocal(rse[:], se_buf[:])
            lg = singles.tile([128, NT], F32)
            nc.vector.tensor_mul(lg[:], lu_buf[:], rse[:])
            emx = singles.tile([128, NT], F32)
            nc.scalar.activation(out=emx[:], in_=lg[:], func=Exp, scale=-1.0)
            z_t = singles.tile([128, NT], F32)
            nc.vector.tensor_scalar_add(z_t[:], emx[:], 1.0)
            rneg = singles.tile([128, NT], F32)
            nc.scalar.activation(out=rneg[:], in_=lg[:], func=Relu, scale=-1.0)
            rpos = singles.tile([128, NT], F32)
            nc.scalar.activation(out=rpos[:], in_=lg[:], func=Relu, scale=1.0)
            absx = singles.tile([128, NT], F32)
            nc.vector.tensor_add(absx[:], rneg[:], rpos[:])
            g0 = singles.tile([128, NT], F32)
            nc.scalar.activation(out=g0[:], in_=absx[:], func=Exp,
                                 scale=-0.7213)
            y_t = singles.tile([128, NT], F32)
            nc.vector.tensor_scalar(out=y_t[:], in0=g0[:], scalar1=LN2,
                                    scalar2=None, op0=mybir.AluOpType.mult)
            nc.vector.tensor_add(y_t[:], y_t[:], rneg[:])
            for step in range(2):
                e_n = singles.tile([128, NT], F32, name=f"e_n{step}")
                nc.scalar.activation(out=e_n[:], in_=y_t[:], func=Exp,
                                     scale=-1.0)
                t_n = singles.tile([128, NT], F32, name=f"t_n{step}")
                nc.vector.tensor_mul(t_n[:], z_t[:], e_n[:])
                y2 = singles.tile([128, NT], F32, name=f"y2_{step}")
                nc.vector.tensor_scalar(out=y2[:], in0=t_n[:], scalar1=-1.0,
                                        scalar2=None, op0=mybir.AluOpType.add)
                nc.vector.tensor_add(y2[:], y2[:], y_t[:])
                y_t = y2
            part = singles.tile([128, 1], F32)
            nc.vector.reduce_sum(out=part[:], in_=y_t[:],
                                 axis=mybir.AxisListType.X)
            ones_t = singles.tile([128, 1], F32)
            nc.vector.memset(ones_t[:], 1.0)
            tot = psum_pool.tile([128, M], F32, tag="s")
            nc.tensor.matmul(tot[0:1, 0:1], part[:], ones_t[:],
                             start=True, stop=True)
            out_sb = singles.tile([1, 1], F32)
            nc.scalar.mul(out_sb[:], tot[0:1, 0:1], -1.0)
            nc.sync.dma_start(out=out_ap, in_=out_sb[:])

    nc.compile()
    return nc


_NC_CACHE = None


def _get_program():
    global _NC_CACHE
    if _NC_CACHE is None:
        _NC_CACHE = _build_program()
    return _NC_CACHE


def _pack_inputs(D, S, W):
    """Host-side shard + transpose-pack + bf16 cast. Returns per-core input maps."""
    bf = ml_dtypes.bfloat16
    Db = D.astype(bf)
    Sb = S.astype(bf)
    Wb = W.astype(bf)
    # stp[p, ds, j] = S[j, ds*128+p]
    stp = np.ascontiguousarray(Sb.reshape(M, KT, 128).transpose(2, 1, 0))
    # wtp[dt, p, es, ii] = W[dt*128+ii, es*128+p]
    wtp = np.ascontiguousarray(
        Wb.reshape(KT, 128, KT, 128).transpose(0, 3, 2, 1))
    in_maps = []
    for c in range(N_CORES):
        Dc = Db[c * N_LOC:(c + 1) * N_LOC]
        # dtp[it, p, ds, ii] = D_shard[it*128+ii, ds*128+p]
        dtp = np.ascontiguousarray(
            Dc.reshape(NT, 128, KT, 128).transpose(0, 3, 2, 1))
        in_maps.append({"dtp": dtp, "stp": stp, "wtp": wtp})
    return in_maps


def kernel(D: np.ndarray, S: np.ndarray, W: np.ndarray) -> np.ndarray:
    assert D.shape == (N_FULL, DS) and S.shape == (M, DS) and W.shape == (DS, DS)
    nc = _get_program()
    in_maps = _pack_inputs(np.asarray(D), np.asarray(S), np.asarray(W))
    res = run_bass_kernel_spmd(nc, in_maps, core_ids=list(range(N_CORES)))
    total = np.float64(0.0)
    for r in res.results:
        total += np.float64(r["out"][0, 0])
    return np.array(total, dtype=np.float32)


# revision 5
# speedup vs baseline: 1.5018x; 1.5018x over previous
"""Trainium2 Bass kernel for nn_ChannelModel (cross-attention + bilinear + logsigmoid sum).

Reference computation (full problem, N=16384, M=1024, Ds=2048):
    scores = (D @ S.T) / sqrt(Ds)            # [N, M]
    w      = softmax(scores, axis=1)         # [N, M]
    att_S  = w @ S                           # [N, Ds]
    logits[i] = D[i] . (W @ att_S[i])        # [N]
    out    = sum(log_sigmoid(logits))        # scalar

Algebraic restructuring:
    logits[i] = (sum_j e_ij * B[i,j]) / (sum_j e_ij)
    with  e = exp(scores/sqrt(Ds)),  B = D @ G.T,  G.T = W @ S.T
which removes the att_S matmul and the big bilinear matmul entirely.

Distribution over 8 cores: D row-sharded (2048 rows/core), S replicated,
GT = W @ S.T computed sharded (2 of 16 row-tiles per core, selected by
feeding each core its own W slice) and AllGathered. Final scalar summed
on host. All matmul inputs bf16 (fp32 PSUM accumulation); validated
end-to-end error vs the fp32 reference is ~1.4e-5 relative.

Phases per core:
    A: GT shard (64 matmuls) -> DRAM -> AllGather -> SBUF
    B: scores + exp for all 16 row-tiles (512 matmuls), e cached in SBUF
    C: B-matmul + e*B row-reduction for all row-tiles (512 matmuls)
    epilogue: logits = lu/se; sum softplus(-logits) via Exp-table Newton
              (no Softplus/Ln ACT table exists on this build)
"""

import math
import os
import sys

for _p in ("/opt/trn_rl_repo", "/root/.axon_site/_ro/trn_rl_repo"):
    if os.path.isdir(_p) and _p not in sys.path:
        sys.path.insert(0, _p)

import ml_dtypes
import numpy as np

import concourse.bass as bass
import concourse.tile as tile
from concourse import bacc, mybir
from concourse.bass_utils import run_bass_kernel_spmd

N_CORES = 8
N_FULL = 16384
M = 1024
DS = 2048
N_LOC = N_FULL // N_CORES   # 2048 rows per core
NT = N_LOC // 128           # 16 row-tiles per core
KT = DS // 128              # 16 contraction slices
JT = M // 512               # 2 free-dim halves of the M axis
LT = KT // N_CORES          # 2 GT row-tiles computed per core

BF16 = mybir.dt.bfloat16
F32 = mybir.dt.float32


def _build_program():
    nc = bacc.Bacc("TRN2", target_bir_lowering=False, debug=False,
                   num_devices=N_CORES)

    # DRAM parameters (per-core shapes; packed on host, see kernel()).
    # dtp[it, p, ds, ii] = D_shard[it*128+ii, ds*128+p]
    dt_ap = nc.dram_tensor("dtp", [NT, 128, KT, 128], BF16,
                           kind="ExternalInput").ap()
    # stp[p, ds, j] = S[j, ds*128+p]   (= S.T with the Ds axis on partitions)
    st_ap = nc.dram_tensor("stp", [128, KT, M], BF16,
                           kind="ExternalInput").ap()
    # wtp[l, p, es, ii] = W[(2c+l)*128+ii, es*128+p]  (core c's 2 GT tiles)
    wt_ap = nc.dram_tensor("wtp", [LT, 128, KT, 128], BF16,
                           kind="ExternalInput").ap()
    out_ap = nc.dram_tensor("out", [1, 1], F32, kind="ExternalOutput").ap()

    scale = 1.0 / math.sqrt(DS)
    Exp = mybir.ActivationFunctionType.Exp
    Relu = mybir.ActivationFunctionType.Relu

    with tile.TileContext(nc) as tc:
        with (
            tc.tile_pool(name="singles", bufs=1) as singles,
            tc.tile_pool(name="wt_pool", bufs=2) as wt_pool,
            tc.tile_pool(name="dt_pool", bufs=3) as dt_pool,
            tc.tile_pool(name="prod_pool", bufs=2) as prod_pool,
            tc.tile_pool(name="psum", bufs=4, space="PSUM") as psum_pool,
            tc.tile_pool(name="dram", bufs=1, space="DRAM") as dram,
        ):
            # Long-lived SBUF tensors.
            st_sb = singles.tile([128, KT, M], BF16)
            for es in range(KT):  # chunked so phase A starts immediately
                nc.sync.dma_start(out=st_sb[:, es, :], in_=st_ap[:, es, :])
            gt_sb = singles.tile([128, KT, M], BF16)
            e_all = singles.tile([128, NT, M], BF16)
            se_buf = singles.tile([128, NT], F32)
            lu_buf = singles.tile([128, NT], F32)

            # ---- Phase A: GT shard (2 row-tiles) + AllGather ----
            gt_loc = singles.tile([128, LT, M], BF16)
            for li in range(LT):
                wt_t = wt_pool.tile([128, KT, 128], BF16, tag="wt")
                nc.sync.dma_start(out=wt_t[:], in_=wt_ap[li])
                pg = psum_pool.tile([128, M], F32, tag="s")
                for jh in range(JT):
                    js = slice(jh * 512, (jh + 1) * 512)
                    for es in range(KT):
                        nc.tensor.matmul(
                            pg[:, js], wt_t[:, es, :], st_sb[:, es, js],
                            start=(es == 0), stop=(es == KT - 1),
                        )
                nc.vector.tensor_copy(gt_loc[:, li, :], pg[:])
            gt_shard = dram.tile([LT, 128, M], BF16)
            gt_all = dram.tile([KT, 128, M], BF16, addr_space="Shared")
            for li in range(LT):
                nc.sync.dma_start(out=gt_shard[li], in_=gt_loc[:, li, :])
            nc.gpsimd.collective_compute(
                "AllGather", mybir.AluOpType.bypass,
                replica_groups=[list(range(N_CORES))],
                ins=[gt_shard.opt()], outs=[gt_all.opt()],
            )
            for dt in range(KT):
                nc.sync.dma_start(out=gt_sb[:, dt, :], in_=gt_all[dt])

            # ---- Phase B: scores + exp for all row-tiles ----
            for it in range(NT):
                dt_t = dt_pool.tile([128, KT, 128], BF16, tag="dt",
                                    name=f"dtb{it}")
                nc.gpsimd.dma_start(out=dt_t[:], in_=dt_ap[it])
                ps = psum_pool.tile([128, M], F32, tag="s")
                for ds in range(KT):
                    lhsT = dt_t[:, ds, :]
                    for jh in range(JT):
                        js = slice(jh * 512, (jh + 1) * 512)
                        nc.tensor.matmul(ps[:, js], lhsT, st_sb[:, ds, js],
                                         start=(ds == 0), stop=(ds == KT - 1))
                nc.scalar.activation(
                    out=e_all[:, it, :], in_=ps[:], func=Exp,
                    scale=scale, accum_out=se_buf[:, it:it + 1],
                )

            # ---- Phase C: B = D @ G.T, lu = rowsum(e * B) ----
            for it in range(NT):
                dt_t = dt_pool.tile([128, KT, 128], BF16, tag="dt",
                                    name=f"dtc{it}")
                nc.gpsimd.dma_start(out=dt_t[:], in_=dt_ap[it])
                pb = psum_pool.tile([128, M], F32, tag="s")
                for ds in range(KT):
                    lhsT = dt_t[:, ds, :]
                    for jh in range(JT):
                        js = slice(jh * 512, (jh + 1) * 512)
                        nc.tensor.matmul(pb[:, js], lhsT, gt_sb[:, ds, js],
                                         start=(ds == 0), stop=(ds == KT - 1))
                prod_t = prod_pool.tile([128, M], F32, tag="p")
                nc.vector.tensor_mul(prod_t[:], pb[:], e_all[:, it, :])
                nc.vector.reduce_sum(lu_buf[:, it:it + 1], prod_t[:],
                                     mybir.AxisListType.X)

            # ---- Epilogue: logits -> sum(softplus(-logits)) -> scalar ----
            # softplus(-x) = ln(z), z = 1 + exp(-x), initial guess
            # relu(-x) + ln2*exp(-0.7213*|x|), then 2 Newton steps
            # y <- y - 1 + z*exp(-y). Stays within the Exp/Relu/Copy table.
            LN2 = 0.6931471805599453
            rse = singles.tile([128, NT], F32)
            nc.vector.reciprocal(rse[:], se_buf[:])
            lg = singles.tile([128, NT], F32)
            nc.vector.tensor_mul(lg[:], lu_buf[:], rse[:])
            emx = singles.tile([128, NT], F32)
            nc.scalar.activation(out=emx[:], in_=lg[:], func=Exp, scale=-1.0)
            z_t = singles.tile([128, NT], F32)
            nc.vector.tensor_scalar_add(z_t[:], emx[:], 1.0)
            rneg = singles.tile([128, NT], F32)
            nc.scalar.activation(out=rneg[:], in_=lg[:], func=Relu, scale=-1.0)
            rpos = singles.tile([128, NT], F32)
            nc.scalar.activation(out=rpos[:], in_=lg[:], func=Relu, scale=1.0)
            absx = singles.tile([128, NT], F32)
            nc.vector.tensor_add(absx[:], rneg[:], rpos[:])
            g0 = singles.tile([128, NT], F32)
            nc.scalar.activation(out=g0[:], in_=absx[:], func=Exp,
                                 scale=-0.7213)
            y_t = singles.tile([128, NT], F32)
            nc.vector.tensor_scalar(out=y_t[:], in0=g0[:], scalar1=LN2,
                                    scalar2=None, op0=mybir.AluOpType.mult)
            nc.vector.tensor_add(y_t[:], y_t[:], rneg[:])
            for step in range(2):
                e_n = singles.tile([128, NT], F32, name=f"e_n{step}")
                nc.scalar.activation(out=e_n[:], in_=y_t[:], func=Exp,
                                     scale=-1.0)
                t_n = singles.tile([128, NT], F32, name=f"t_n{step}")
                nc.vector.tensor_mul(t_n[:], z_t[:], e_n[:])
                y2 = singles.tile([128, NT], F32, name=f"y2_{step}")
                nc.vector.tensor_scalar(out=y2[:], in0=t_n[:], scalar1=-1.0,
                                        scalar2=None, op0=mybir.AluOpType.add)
                nc.vector.tensor_add(y2[:], y2[:], y_t[:])
                y_t = y2
            part = singles.tile([128, 1], F32)
            nc.vector.reduce_sum(out=part[:], in_=y_t[:],
                                 axis=mybir.AxisListType.X)
            ones_t = singles.tile([128, 1], F32)
            nc.vector.memset(ones_t[:], 1.0)
            tot = psum_pool.tile([128, M], F32, tag="s")
            nc.tensor.matmul(tot[0:1, 0:1], part[:], ones_t[:],
                             start=True, stop=True)
            out_sb = singles.tile([1, 1], F32)
            nc.scalar.mul(out_sb[:], tot[0:1, 0:1], -1.0)
            nc.sync.dma_start(out=out_ap, in_=out_sb[:])

    nc.compile()
    return nc


_NC_CACHE = None


def _get_program():
    global _NC_CACHE
    if _NC_CACHE is None:
        _NC_CACHE = _build_program()
    return _NC_CACHE


def _pack_inputs(D, S, W):
    """Host-side shard + transpose-pack + bf16 cast. Returns per-core input maps."""
    bf = ml_dtypes.bfloat16
    Db = D.astype(bf)
    Sb = S.astype(bf)
    Wb = W.astype(bf)
    # stp[p, ds, j] = S[j, ds*128+p]
    stp = np.ascontiguousarray(Sb.reshape(M, KT, 128).transpose(2, 1, 0))
    # wtp_full[dt, p, es, ii] = W[dt*128+ii, es*128+p]
    wtp_full = np.ascontiguousarray(
        Wb.reshape(KT, 128, KT, 128).transpose(0, 3, 2, 1))
    in_maps = []
    for c in range(N_CORES):
        Dc = Db[c * N_LOC:(c + 1) * N_LOC]
        # dtp[it, p, ds, ii] = D_shard[it*128+ii, ds*128+p]
        dtp = np.ascontiguousarray(
            Dc.reshape(NT, 128, KT, 128).transpose(0, 3, 2, 1))
        wtp = np.ascontiguousarray(wtp_full[c * LT:(c + 1) * LT])
        in_maps.append({"dtp": dtp, "stp": stp, "wtp": wtp})
    return in_maps


def kernel(D: np.ndarray, S: np.ndarray, W: np.ndarray) -> np.ndarray:
    assert D.shape == (N_FULL, DS) and S.shape == (M, DS) and W.shape == (DS, DS)
    nc = _get_program()
    in_maps = _pack_inputs(np.asarray(D), np.asarray(S), np.asarray(W))
    res = run_bass_kernel_spmd(nc, in_maps, core_ids=list(range(N_CORES)))
    total = np.float64(0.0)
    for r in res.results:
        total += np.float64(r["out"][0, 0])
    return np.array(total, dtype=np.float32)


# revision 19
# speedup vs baseline: 1.5241x; 1.0149x over previous
"""Trainium2 Bass kernel for nn_ChannelModel (cross-attention + bilinear + logsigmoid sum).

Reference computation (full problem, N=16384, M=1024, Ds=2048):
    scores = (D @ S.T) / sqrt(Ds)            # [N, M]
    w      = softmax(scores, axis=1)         # [N, M]
    att_S  = w @ S                           # [N, Ds]
    logits[i] = D[i] . (W @ att_S[i])        # [N]
    out    = sum(log_sigmoid(logits))        # scalar

Algebraic restructuring:
    logits[i] = (sum_j e_ij * B[i,j]) / (sum_j e_ij)
    with  e = exp(scores/sqrt(Ds)),  B = D @ G.T,  G.T = W @ S.T
which removes the att_S matmul and the big bilinear matmul entirely.

Distribution over 8 cores: D row-sharded (2048 rows/core), S replicated,
GT = W @ S.T computed sharded (2 of 16 row-tiles per core, selected by
feeding each core its own W slice) and AllGathered (fp8, 2MB). Final
scalar summed on host.

Precision: phase A (GT) runs bf16; the two big phases run fp8e4m3 with
DoubleRow (2 fp8 rows per PE cell -> K=256 per matmul instruction),
fp32 PSUM accumulation throughout. Validated end-to-end error of the
final scalar vs the fp32 reference: ~1e-4 relative (threshold 2e-2) —
the final sum averages 16384 logits, so elementwise quantization noise
cancels.

Phases per core:
    A: GT shard (64 bf16 matmuls) -> fp8 -> AllGather -> SBUF
    B: scores + exp for all 16 row-tiles (256 DoubleRow matmuls),
       e cached in SBUF (bf16)
    C: B-matmul + e*B row-reduction (256 DoubleRow matmuls)
    epilogue: logits = lu/se; sum softplus(-logits) via an Exp-table
       Newton iteration (no Softplus/Ln ACT table exists on this build),
       first half emitted mid-phase-C on ACT/GpSimd so it overlaps.
"""

import math
import os
import sys

for _p in ("/opt/trn_rl_repo", "/root/.axon_site/_ro/trn_rl_repo"):
    if os.path.isdir(_p) and _p not in sys.path:
        sys.path.insert(0, _p)

import ml_dtypes
import numpy as np

import concourse.bass as bass
import concourse.tile as tile
from concourse import bacc, mybir
from concourse.bass_utils import run_bass_kernel_spmd

N_CORES = 8
N_FULL = 16384
M = 1024
DS = 2048
N_LOC = N_FULL // N_CORES   # 2048 rows per core
NT = N_LOC // 128           # 16 row-tiles per core
KT = DS // 128              # 16 contraction slices
PT = KT // 2                # 8 DoubleRow contraction pair-slices
JT = M // 512               # 2 free-dim halves of the M axis
LT = KT // N_CORES          # 2 GT row-tiles computed per core

BF16 = mybir.dt.bfloat16
FP8 = mybir.dt.float8e4
F32 = mybir.dt.float32
DR = mybir.MatmulPerfMode.DoubleRow


def _build_program():
    nc = bacc.Bacc("TRN2", target_bir_lowering=False, debug=False,
                   num_devices=N_CORES)

    # DRAM parameters (per-core shapes; packed on host, see kernel()).
    # dtp[it, p, ds, ii] = D_shard[it*128+ii, ds*128+p]   (fp8)
    dt_ap = nc.dram_tensor("dtp", [NT, 128, KT, 128], FP8,
                           kind="ExternalInput").ap()
    # stp8[p, ds, j] = S[j, ds*128+p]  (= S.T, Ds on partitions; fp8)
    st8_ap = nc.dram_tensor("stp8", [128, KT, M], FP8,
                            kind="ExternalInput").ap()
    # stpb: same layout in bf16 (phase A rhs)
    stb_ap = nc.dram_tensor("stpb", [128, KT, M], BF16,
                            kind="ExternalInput").ap()
    # wtp[l, p, es, ii] = W[(2c+l)*128+ii, es*128+p]  (core c's 2 GT tiles)
    wt_ap = nc.dram_tensor("wtp", [LT, 128, KT, 128], BF16,
                           kind="ExternalInput").ap()
    out_ap = nc.dram_tensor("out", [1, 1], F32, kind="ExternalOutput").ap()

    scale = 1.0 / math.sqrt(DS)
    Exp = mybir.ActivationFunctionType.Exp
    Relu = mybir.ActivationFunctionType.Relu

    with tile.TileContext(nc) as tc:
        with (
            tc.tile_pool(name="singles", bufs=1) as singles,
            tc.tile_pool(name="wt_pool", bufs=2) as wt_pool,
            tc.tile_pool(name="dt_pool", bufs=16) as dt_pool,
            tc.tile_pool(name="prod_pool", bufs=2) as prod_pool,
            tc.tile_pool(name="psum", bufs=4, space="PSUM") as psum_pool,
            tc.tile_pool(name="dram", bufs=1, space="DRAM") as dram,
        ):
            # Long-lived SBUF tensors.
            stb_sb = singles.tile([128, KT, M], BF16)
            st8_sb = singles.tile([128, KT, M], FP8)
            gt8_sb = singles.tile([128, KT, M], FP8)
            e_all = singles.tile([128, NT, M], BF16)
            se_buf = singles.tile([128, NT], F32)
            lu_buf = singles.tile([128, NT], F32)

            # ---- Phase A: GT shard (2 row-tiles, bf16) + fp8 AllGather ----
            # wt0 first, then bf16 st in per-slice chunks so the first GT
            # matmul starts ~3us in; everything downstream of phase A (the
            # AllGather above all) shifts earlier by the same amount.
            gt_loc = singles.tile([128, LT, M], FP8)
            gt_shard = dram.tile([LT, 128, M], FP8)
            gt_all = dram.tile([KT, 128, M], FP8, addr_space="Shared")
            wt0 = wt_pool.tile([128, KT, 128], BF16, tag="wt", name="wt0")
            wt1 = wt_pool.tile([128, KT, 128], BF16, tag="wt", name="wt1")
            nc.sync.dma_start(out=wt0[:], in_=wt_ap[0])
            for es in range(KT):
                nc.sync.dma_start(out=stb_sb[:, es, :], in_=stb_ap[:, es, :])
            nc.sync.dma_start(out=wt1[:], in_=wt_ap[1])
            nc.sync.dma_start(out=st8_sb[:], in_=st8_ap)
            for li, wt_t in ((0, wt0), (1, wt1)):
                pg = psum_pool.tile([128, M], F32, tag="s")
                for jh in range(JT):
                    js = slice(jh * 512, (jh + 1) * 512)
                    for es in range(KT):
                        nc.tensor.matmul(
                            pg[:, js], wt_t[:, es, :], stb_sb[:, es, js],
                            start=(es == 0), stop=(es == KT - 1),
                        )
                nc.vector.tensor_copy(gt_loc[:, li, :], pg[:])
                nc.sync.dma_start(out=gt_shard[li], in_=gt_loc[:, li, :])
            # gpsimd hosts ONLY the collective (SWDGE DMAs there would
            # serialize behind it; sync-engine collectives hang in NRT).
            nc.gpsimd.collective_compute(
                "AllGather", mybir.AluOpType.bypass,
                replica_groups=[list(range(N_CORES))],
                ins=[gt_shard.opt()], outs=[gt_all.opt()],
            )

            # ---- Phase B: scores + exp for all row-tiles (fp8 DoubleRow) ----
            # dt tiles stay resident for reuse in phase C (16 x 2KB/partition).
            dts = []
            for it in range(NT):
                dt_t = dt_pool.tile([128, KT, 128], FP8, tag="dt",
                                    name=f"dtb{it}")
                nc.sync.dma_start(out=dt_t[:], in_=dt_ap[it])
                dts.append(dt_t)
                ps = psum_pool.tile([128, M], F32, tag="s")
                for k in range(PT):
                    ks = slice(2 * k, 2 * k + 2)
                    for jh in range(JT):
                        js = slice(jh * 512, (jh + 1) * 512)
                        nc.tensor.matmul(ps[:, js], dt_t[:, ks, :],
                                         st8_sb[:, ks, js], perf_mode=DR,
                                         start=(k == 0), stop=(k == PT - 1))
                nc.scalar.activation(
                    out=e_all[:, it, :], in_=ps[:], func=Exp,
                    scale=scale, accum_out=se_buf[:, it:it + 1],
                )

            # GT gather lands here: emitted after phase B's dt loads so the
            # sync DMA queue never blocks them behind the collective wait.
            # gather block g holds core g's dtiles (g*LT .. g*LT+LT-1).
            for dt in range(KT):
                nc.sync.dma_start(out=gt8_sb[:, dt, :], in_=gt_all[dt])

            # Epilogue math: logits = lu/se, then sum softplus(-logits).
            # softplus(-x) = ln(z), z = 1 + exp(-x), initial guess
            # relu(-x) + ln2*exp(-0.7213*|x|), then 2 Newton steps
            # y <- y - 1 + z*exp(-y). Stays within the Exp/Relu/Copy table.
            # Elementwise ops go to GpSimd (DVE is busy with phase C and is
            # strict FIFO; GpSimd is idle once the collective is done); the
            # row-sum uses the ACT accumulator. Half 0 is emitted mid-phase-C
            # so its serial chain overlaps; only two tiny matmuls run at the
            # very end.
            LN2 = 0.6931471805599453
            NH = NT // 2
            parts = []

            def epilogue_half(h):
                hs = slice(h * NH, (h + 1) * NH)

                def ht(name):
                    return singles.tile([128, NH], F32, name=f"{name}_h{h}")

                rse = ht("rse")
                nc.vector.reciprocal(rse[:], se_buf[:, hs])
                lg = ht("lg")
                nc.gpsimd.tensor_mul(lg[:], lu_buf[:, hs], rse[:])
                emx = ht("emx")
                nc.scalar.activation(out=emx[:], in_=lg[:], func=Exp,
                                     scale=-1.0)
                z_t = ht("z_t")
                nc.gpsimd.tensor_scalar_add(z_t[:], emx[:], 1.0)
                rneg = ht("rneg")
                nc.scalar.activation(out=rneg[:], in_=lg[:], func=Relu,
                                     scale=-1.0)
                rpos = ht("rpos")
                nc.scalar.activation(out=rpos[:], in_=lg[:], func=Relu,
                                     scale=1.0)
                absx = ht("absx")
                nc.gpsimd.tensor_add(absx[:], rneg[:], rpos[:])
                g0 = ht("g0")
                nc.scalar.activation(out=g0[:], in_=absx[:], func=Exp,
                                     scale=-0.7213)
                y_t = ht("y0")
                nc.gpsimd.tensor_scalar(out=y_t[:], in0=g0[:], scalar1=LN2,
                                        scalar2=None,
                                        op0=mybir.AluOpType.mult)
                nc.gpsimd.tensor_add(y_t[:], y_t[:], rneg[:])
                for step in range(2):
                    e_n = ht(f"e_n{step}")
                    nc.scalar.activation(out=e_n[:], in_=y_t[:], func=Exp,
                                         scale=-1.0)
                    t_n = ht(f"t_n{step}")
                    nc.gpsimd.tensor_mul(t_n[:], z_t[:], e_n[:])
                    y2 = ht(f"y2_{step}")
                    nc.gpsimd.tensor_scalar(out=y2[:], in0=t_n[:],
                                            scalar1=-1.0, scalar2=None,
                                            op0=mybir.AluOpType.add)
                    nc.gpsimd.tensor_add(y2[:], y2[:], y_t[:])
                    y_t = y2
                part = ht("part")
                ysc = ht("ysc")
                # free-dim row-sum via the ACT accumulator (gpsimd can only
                # reduce along partitions; DVE would block phase C's queue)
                nc.scalar.activation(
                    out=ysc[:], in_=y_t[:],
                    func=mybir.ActivationFunctionType.Identity,
                    accum_out=part[:, 0:1])
                parts.append(part)

            # ---- Phase C: B = D @ G.T, lu = rowsum(e * B), fp8 DoubleRow ----
            for it in range(NT):
                dt_t = dts[it]
                pb = psum_pool.tile([128, M], F32, tag="s")
                for k in range(PT):
                    ks = slice(2 * k, 2 * k + 2)
                    for jh in range(JT):
                        js = slice(jh * 512, (jh + 1) * 512)
                        nc.tensor.matmul(pb[:, js], dt_t[:, ks, :],
                                         gt8_sb[:, ks, js], perf_mode=DR,
                                         start=(k == 0), stop=(k == PT - 1))
                prod_t = prod_pool.tile([128, M], F32, tag="p")
                nc.vector.tensor_mul(prod_t[:], pb[:], e_all[:, it, :])
                nc.vector.reduce_sum(lu_buf[:, it:it + 1], prod_t[:],
                                     mybir.AxisListType.X)
                if it == NH - 1:
                    epilogue_half(0)
            epilogue_half(1)

            ones_t = singles.tile([128, 1], F32)
            nc.vector.memset(ones_t[:], 1.0)
            tot = psum_pool.tile([128, M], F32, tag="s")
            for h in range(2):
                nc.tensor.matmul(tot[0:1, 0:1], parts[h][:, 0:1], ones_t[:],
                                 start=(h == 0), stop=(h == 1))
            out_sb = singles.tile([1, 1], F32)
            nc.scalar.mul(out_sb[:], tot[0:1, 0:1], -1.0)
            nc.sync.dma_start(out=out_ap, in_=out_sb[:])

    nc.compile()
    return nc


_NC_CACHE = None


def _get_program():
    global _NC_CACHE
    if _NC_CACHE is None:
        _NC_CACHE = _build_program()
    return _NC_CACHE


def _pack_inputs(D, S, W):
    """Host-side shard + transpose-pack + cast. Returns per-core input maps."""
    bf = ml_dtypes.bfloat16
    f8 = ml_dtypes.float8_e4m3
    D8 = D.astype(f8)
    Sb = S.astype(bf)
    Wb = W.astype(bf)
    # stp[p, ds, j] = S[j, ds*128+p]
    stpb = np.ascontiguousarray(Sb.reshape(M, KT, 128).transpose(2, 1, 0))
    stp8 = np.ascontiguousarray(
        S.astype(f8).reshape(M, KT, 128).transpose(2, 1, 0))
    # wtp_full[dt, p, es, ii] = W[dt*128+ii, es*128+p]
    wtp_full = np.ascontiguousarray(
        Wb.reshape(KT, 128, KT, 128).transpose(0, 3, 2, 1))
    in_maps = []
    for c in range(N_CORES):
        Dc = D8[c * N_LOC:(c + 1) * N_LOC]
        # dtp[it, p, ds, ii] = D_shard[it*128+ii, ds*128+p]
        dtp = np.ascontiguousarray(
            Dc.reshape(NT, 128, KT, 128).transpose(0, 3, 2, 1))
        wtp = np.ascontiguousarray(wtp_full[c * LT:(c + 1) * LT])
        in_maps.append({"dtp": dtp, "stp8": stp8, "stpb": stpb, "wtp": wtp})
    return in_maps


def kernel(D: np.ndarray, S: np.ndarray, W: np.ndarray) -> np.ndarray:
    assert D.shape == (N_FULL, DS) and S.shape == (M, DS) and W.shape == (DS, DS)
    nc = _get_program()
    in_maps = _pack_inputs(np.asarray(D), np.asarray(S), np.asarray(W))
    res = run_bass_kernel_spmd(nc, in_maps, core_ids=list(range(N_CORES)))
    total = np.float64(0.0)
    for r in res.results:
        total += np.float64(r["out"][0, 0])
    return np.array(total, dtype=np.float32)


# revision 20
# speedup vs baseline: 1.5928x; 1.0451x over previous
"""Trainium2 Bass kernel for nn_ChannelModel (cross-attention + bilinear + logsigmoid sum).

Reference computation (full problem, N=16384, M=1024, Ds=2048):
    scores = (D @ S.T) / sqrt(Ds)            # [N, M]
    w      = softmax(scores, axis=1)         # [N, M]
    att_S  = w @ S                           # [N, Ds]
    logits[i] = D[i] . (W @ att_S[i])        # [N]
    out    = sum(log_sigmoid(logits))        # scalar

Algebraic restructuring:
    logits[i] = (sum_j e_ij * B[i,j]) / (sum_j e_ij)
    with  e = exp(scores/sqrt(Ds)),  B = D @ G.T,  G.T = W @ S.T
which removes the att_S matmul and the big bilinear matmul entirely
(34.4 -> 25.8 GFLOP per core, and G.T is tiny enough to recompute).

Distribution over 8 cores: D row-sharded (2048 rows/core), S and W
replicated; each core computes G.T itself (no collectives), partial
logsigmoid sums are added on the host.

Precision: all matmuls run fp8e4m3 inputs with DoubleRow (2 fp8 rows
per PE cell -> K=256 contraction per matmul instruction) and fp32 PSUM
accumulation. W is pre-scaled by 64 on the host so its entries (std
0.01) sit in fp8's normal range; the GT copy un-scales by 1/64. The
softmax exp and all reductions are fp32. Validated end-to-end error of
the final scalar vs the fp32 reference: ~6e-4 relative (threshold
2e-2) — the final sum averages 16384 logits, so elementwise fp8
quantization noise largely cancels.

Phases per core:
    A: GT = (64W) @ S.T / 64, 256 DoubleRow matmuls, fp8 result in SBUF
    B: scores + exp for all 16 row-tiles (256 DoubleRow matmuls),
       e cached in SBUF (bf16)
    C: B-matmul + e*B row-reduction (256 DoubleRow matmuls)
    epilogue: logits = lu/se; sum softplus(-logits) via an Exp-table
       Newton iteration (no Softplus/Ln ACT table exists on this build),
       first half emitted mid-phase-C on ACT/GpSimd so it overlaps.
"""

import math
import os
import sys

for _p in ("/opt/trn_rl_repo", "/root/.axon_site/_ro/trn_rl_repo"):
    if os.path.isdir(_p) and _p not in sys.path:
        sys.path.insert(0, _p)

import ml_dtypes
import numpy as np

import concourse.bass as bass
import concourse.tile as tile
from concourse import bacc, mybir
from concourse.bass_utils import run_bass_kernel_spmd

N_CORES = 8
N_FULL = 16384
M = 1024
DS = 2048
N_LOC = N_FULL // N_CORES   # 2048 rows per core
NT = N_LOC // 128           # 16 row-tiles per core
KT = DS // 128              # 16 contraction slices
PT = KT // 2                # 8 DoubleRow contraction pair-slices
JT = M // 512               # 2 free-dim halves of the M axis

W_SCALE = 64.0              # host pre-scale of W for fp8 normal range

BF16 = mybir.dt.bfloat16
FP8 = mybir.dt.float8e4
F32 = mybir.dt.float32
DR = mybir.MatmulPerfMode.DoubleRow


def _build_program():
    nc = bacc.Bacc("TRN2", target_bir_lowering=False, debug=False,
                   num_devices=N_CORES)

    # DRAM parameters (per-core shapes; packed on host, see kernel()).
    # dtp[it, p, ds, ii] = D_shard[it*128+ii, ds*128+p]
    dt_ap = nc.dram_tensor("dtp", [NT, 128, KT, 128], FP8,
                           kind="ExternalInput").ap()
    # stp8[p, es, j] = S[j, es*128+p]  (= S.T, Ds on partitions)
    st8_ap = nc.dram_tensor("stp8", [128, KT, M], FP8,
                            kind="ExternalInput").ap()
    # wtp[dt, p, es, ii] = 64*W[dt*128+ii, es*128+p]
    wt_ap = nc.dram_tensor("wtp", [KT, 128, KT, 128], FP8,
                           kind="ExternalInput").ap()
    out_ap = nc.dram_tensor("out", [1, 1], F32, kind="ExternalOutput").ap()

    scale = 1.0 / math.sqrt(DS)
    Exp = mybir.ActivationFunctionType.Exp
    Relu = mybir.ActivationFunctionType.Relu

    with tile.TileContext(nc) as tc:
        with (
            tc.tile_pool(name="singles", bufs=1) as singles,
            tc.tile_pool(name="wt_pool", bufs=3) as wt_pool,
            tc.tile_pool(name="dt_pool", bufs=16) as dt_pool,
            tc.tile_pool(name="prod_pool", bufs=2) as prod_pool,
            tc.tile_pool(name="psum", bufs=4, space="PSUM") as psum_pool,
        ):
            # Long-lived SBUF tensors.
            st8_sb = singles.tile([128, KT, M], FP8)
            gt8_sb = singles.tile([128, KT, M], FP8)
            e_all = singles.tile([128, NT, M], BF16)
            se_buf = singles.tile([128, NT], F32)
            lu_buf = singles.tile([128, NT], F32)

            # ---- Phase A: GT = (64W) @ S.T / 64, fp8 DoubleRow ----
            # wt[0] first, then st in per-slice chunks, so the first matmul
            # starts ~2us in; the remaining wt tiles stream during compute.
            wt_first = wt_pool.tile([128, KT, 128], FP8, tag="wt",
                                    name="wt_first")
            nc.sync.dma_start(out=wt_first[:], in_=wt_ap[0])
            for es in range(KT):
                nc.sync.dma_start(out=st8_sb[:, es, :], in_=st8_ap[:, es, :])
            for dt_i in range(KT):
                if dt_i == 0:
                    wt_t = wt_first
                else:
                    wt_t = wt_pool.tile([128, KT, 128], FP8, tag="wt",
                                        name=f"wt{dt_i}")
                    nc.sync.dma_start(out=wt_t[:], in_=wt_ap[dt_i])
                pg = psum_pool.tile([128, M], F32, tag="s")
                for k in range(PT):
                    ks = slice(2 * k, 2 * k + 2)
                    for jh in range(JT):
                        js = slice(jh * 512, (jh + 1) * 512)
                        nc.tensor.matmul(pg[:, js], wt_t[:, ks, :],
                                         st8_sb[:, ks, js], perf_mode=DR,
                                         start=(k == 0), stop=(k == PT - 1))
                # un-scale by 1/64 and quantize to fp8 in one DVE pass
                nc.vector.tensor_scalar_mul(gt8_sb[:, dt_i, :], pg[:],
                                            1.0 / W_SCALE)

            # ---- Phase B: scores + exp for all row-tiles (fp8 DoubleRow) ----
            # dt tiles stay resident for reuse in phase C (16 x 2KB/partition).
            dts = []
            for it in range(NT):
                dt_t = dt_pool.tile([128, KT, 128], FP8, tag="dt",
                                    name=f"dtb{it}")
                nc.sync.dma_start(out=dt_t[:], in_=dt_ap[it])
                dts.append(dt_t)
                ps = psum_pool.tile([128, M], F32, tag="s")
                for k in range(PT):
                    ks = slice(2 * k, 2 * k + 2)
                    for jh in range(JT):
                        js = slice(jh * 512, (jh + 1) * 512)
                        nc.tensor.matmul(ps[:, js], dt_t[:, ks, :],
                                         st8_sb[:, ks, js], perf_mode=DR,
                                         start=(k == 0), stop=(k == PT - 1))
                nc.scalar.activation(
                    out=e_all[:, it, :], in_=ps[:], func=Exp,
                    scale=scale, accum_out=se_buf[:, it:it + 1],
                )

            # Epilogue math: logits = lu/se, then sum softplus(-logits).
            # softplus(-x) = ln(z), z = 1 + exp(-x), initial guess
            # relu(-x) + ln2*exp(-0.7213*|x|), then 2 Newton steps
            # y <- y - 1 + z*exp(-y). Stays within the Exp/Relu/Copy table.
            # Elementwise ops go to GpSimd (DVE is busy with phase C and is
            # strict FIFO); the row-sum uses the ACT accumulator. Half 0 is
            # emitted mid-phase-C so its serial chain overlaps; only two tiny
            # matmuls run at the very end.
            LN2 = 0.6931471805599453
            NH = NT // 2
            parts = []

            def epilogue_half(h):
                hs = slice(h * NH, (h + 1) * NH)

                def ht(name):
                    return singles.tile([128, NH], F32, name=f"{name}_h{h}")

                rse = ht("rse")
                nc.vector.reciprocal(rse[:], se_buf[:, hs])
                lg = ht("lg")
                nc.gpsimd.tensor_mul(lg[:], lu_buf[:, hs], rse[:])
                emx = ht("emx")
                nc.scalar.activation(out=emx[:], in_=lg[:], func=Exp,
                                     scale=-1.0)
                z_t = ht("z_t")
                nc.gpsimd.tensor_scalar_add(z_t[:], emx[:], 1.0)
                rneg = ht("rneg")
                nc.scalar.activation(out=rneg[:], in_=lg[:], func=Relu,
                                     scale=-1.0)
                rpos = ht("rpos")
                nc.scalar.activation(out=rpos[:], in_=lg[:], func=Relu,
                                     scale=1.0)
                absx = ht("absx")
                nc.gpsimd.tensor_add(absx[:], rneg[:], rpos[:])
                g0 = ht("g0")
                nc.scalar.activation(out=g0[:], in_=absx[:], func=Exp,
                                     scale=-0.7213)
                y_t = ht("y0")
                nc.gpsimd.tensor_scalar(out=y_t[:], in0=g0[:], scalar1=LN2,
                                        scalar2=None,
                                        op0=mybir.AluOpType.mult)
                nc.gpsimd.tensor_add(y_t[:], y_t[:], rneg[:])
                for step in range(2):
                    e_n = ht(f"e_n{step}")
                    nc.scalar.activation(out=e_n[:], in_=y_t[:], func=Exp,
                                         scale=-1.0)
                    t_n = ht(f"t_n{step}")
                    nc.gpsimd.tensor_mul(t_n[:], z_t[:], e_n[:])
                    y2 = ht(f"y2_{step}")
                    nc.gpsimd.tensor_scalar(out=y2[:], in0=t_n[:],
                                            scalar1=-1.0, scalar2=None,
                                            op0=mybir.AluOpType.add)
                    nc.gpsimd.tensor_add(y2[:], y2[:], y_t[:])
                    y_t = y2
                part = ht("part")
                ysc = ht("ysc")
                # free-dim row-sum via the ACT accumulator (gpsimd can only
                # reduce along partitions; DVE would block phase C's queue)
                nc.scalar.activation(
                    out=ysc[:], in_=y_t[:],
                    func=mybir.ActivationFunctionType.Identity,
                    accum_out=part[:, 0:1])
                parts.append(part)

            # ---- Phase C: B = D @ G.T, lu = rowsum(e * B), fp8 DoubleRow ----
            for it in range(NT):
                dt_t = dts[it]
                pb = psum_pool.tile([128, M], F32, tag="s")
                for k in range(PT):
                    ks = slice(2 * k, 2 * k + 2)
                    for jh in range(JT):
                        js = slice(jh * 512, (jh + 1) * 512)
                        nc.tensor.matmul(pb[:, js], dt_t[:, ks, :],
                                         gt8_sb[:, ks, js], perf_mode=DR,
                                         start=(k == 0), stop=(k == PT - 1))
                prod_t = prod_pool.tile([128, M], F32, tag="p")
                nc.vector.tensor_mul(prod_t[:], pb[:], e_all[:, it, :])
                nc.vector.reduce_sum(lu_buf[:, it:it + 1], prod_t[:],
                                     mybir.AxisListType.X)
                if it == NH - 1:
                    epilogue_half(0)
            epilogue_half(1)

            ones_t = singles.tile([128, 1], F32)
            nc.vector.memset(ones_t[:], 1.0)
            tot = psum_pool.tile([128, M], F32, tag="s")
            for h in range(2):
                nc.tensor.matmul(tot[0:1, 0:1], parts[h][:, 0:1], ones_t[:],
                                 start=(h == 0), stop=(h == 1))
            out_sb = singles.tile([1, 1], F32)
            nc.scalar.mul(out_sb[:], tot[0:1, 0:1], -1.0)
            nc.sync.dma_start(out=out_ap, in_=out_sb[:])

    nc.compile()
    return nc


_NC_CACHE = None


def _get_program():
    global _NC_CACHE
    if _NC_CACHE is None:
        _NC_CACHE = _build_program()
    return _NC_CACHE


def _pack_inputs(D, S, W):
    """Host-side shard + transpose-pack + fp8 cast. Returns per-core maps."""
    f8 = ml_dtypes.float8_e4m3
    D8 = D.astype(f8)
    # stp8[p, es, j] = S[j, es*128+p]
    stp8 = np.ascontiguousarray(
        S.astype(f8).reshape(M, KT, 128).transpose(2, 1, 0))
    # wtp[dt, p, es, ii] = 64*W[dt*128+ii, es*128+p]
    W64 = (W.astype(np.float32) * W_SCALE).astype(f8)
    wtp = np.ascontiguousarray(
        W64.reshape(KT, 128, KT, 128).transpose(0, 3, 2, 1))
    in_maps = []
    for c in range(N_CORES):
        Dc = D8[c * N_LOC:(c + 1) * N_LOC]
        # dtp[it, p, ds, ii] = D_shard[it*128+ii, ds*128+p]
        dtp = np.ascontiguousarray(
            Dc.reshape(NT, 128, KT, 128).transpose(0, 3, 2, 1))
        in_maps.append({"dtp": dtp, "stp8": stp8, "wtp": wtp})
    return in_maps


def kernel(D: np.ndarray, S: np.ndarray, W: np.ndarray) -> np.ndarray:
    assert D.shape == (N_FULL, DS) and S.shape == (M, DS) and W.shape == (DS, DS)
    nc = _get_program()
    in_maps = _pack_inputs(np.asarray(D), np.asarray(S), np.asarray(W))
    res = run_bass_kernel_spmd(nc, in_maps, core_ids=list(range(N_CORES)))
    total = np.float64(0.0)
    for r in res.results:
        total += np.float64(r["out"][0, 0])
    return np.array(total, dtype=np.float32)
